# revision 6
# baseline (speedup 1.0000x reference)
"""Causal self-attention (B=2, S=2048, D=2048, H=16) on 8 trn2 NeuronCores.

Sharding: core c -> batch b = c//4, head-group hg = c%4 (4 heads of 128 dims).
Each core computes its heads' attention plus the partial output projection
(row-parallel split of W_proj); the host sums the 4 partials per batch.

Mixed precision built around fp8e4 DoubleRow matmuls (2 fp8 weights per PE
cell -> 256-deep contraction at 0.5 cycles/row):
- Q/K/V projections: fp8 DR (weights prescaled x64, evac copy scales 1/64)
- scores: bf16 (same PE rate as fp32r, half the SBUF traffic)
- PV + softmax denominators: fp8 DR over e8 = exp tiles written as fp8
- output projection: fp8 DR (a prescaled x16, evac scales 1/1024)
Early rows are precision-critical (softmax concentrates on few keys), so
queries 0-127 (which only see keys 0-127) run a bf16 path end-to-end:
bf16 V for keys 0-127, bf16 e/ups/denominator, and a bf16 m-tile-0 output
projection. Everything stays in SBUF; no DRAM scratch round trips.
"""

import sys

sys.path.insert(0, "/opt/trn_rl_repo")

from contextlib import ExitStack

import numpy as np
import ml_dtypes

import concourse.bass as bass
import concourse.mybir as mybir
import concourse.tile as tile
from concourse import bacc
from concourse.bass_utils import run_bass_kernel_spmd

B, S, D, H = 2, 2048, 2048, 16
HD = D // H  # 128
NH = 4  # heads per core
HG = H // NH  # head groups = 4
P = 128
KT = D // P  # 16 k-tiles over model dim
NQ = 4  # q-groups of 512
QW = S // NQ  # 512
ST = S // P  # 16 token-tiles of 128
SCALE = float(1.0 / np.sqrt(D).astype(np.float32))
MASK_NEG = -30000.0  # exp(SCALE * -30000) == 0 in fp32
WS = 64.0  # weight prescale for fp8
AS = 16.0  # attention-out prescale for fp8

F32 = mybir.dt.float32
BF = mybir.dt.bfloat16
F8 = mybir.dt.float8e4
DR = mybir.MatmulPerfMode.DoubleRow
ADD = mybir.AluOpType.add
MULT = mybir.AluOpType.mult
EXP = mybir.ActivationFunctionType.Exp
COPY = mybir.ActivationFunctionType.Copy

F8NP = ml_dtypes.float8_e4m3
BFNP = ml_dtypes.bfloat16


def build_bass():
    nc = bacc.Bacc("TRN2")

    x8 = nc.declare_dram_parameter("x8", [P, KT, S], F8, isOutput=False)
    wq8 = nc.declare_dram_parameter("wq8", [P, KT, NH * HD], F8, isOutput=False)
    wk8 = nc.declare_dram_parameter("wk8", [P, KT, NH * HD], F8, isOutput=False)
    wv8 = nc.declare_dram_parameter("wv8", [P, KT, NH * HD], F8, isOutput=False)
    wv16 = nc.declare_dram_parameter("wv16", [P, KT, NH * HD], BF, isOutput=False)
    xbT16 = nc.declare_dram_parameter("xbT16", [P, KT, P], BF, isOutput=False)
    wp8 = nc.declare_dram_parameter("wp8", [P, NH, D], F8, isOutput=False)
    wp16 = nc.declare_dram_parameter("wp16", [P, NH, D], BF, isOutput=False)
    mask = nc.declare_dram_parameter("mask", [P, 3 * P], F32, isOutput=False)
    y = nc.declare_dram_parameter("y", [S, D], F32, isOutput=True)

    with tile.TileContext(nc) as tc, ExitStack() as top:
        const = top.enter_context(tc.tile_pool(name="const", bufs=1))
        main = top.enter_context(tc.tile_pool(name="main", bufs=1))

        # ---- constants ----
        mask_sb = const.tile([P, 3 * P], F32)
        nc.gpsimd.dma_start(mask_sb, mask[:, :])
        ones8_t = const.tile([P, 2, 16], F8)
        nc.vector.memset(ones8_t, 1.0)
        ones8 = ones8_t[:, :, 0:1]  # DR lhsT: pair step 16B (ISA: step%16==0)
        ones16_t = const.tile([P, 16], BF)
        nc.vector.memset(ones16_t, 1.0)
        ones16 = ones16_t[:, 0:1]

        # ---- persistent tensors ----
        x8_sb = main.tile([P, KT, S], F8)
        wq8_sb = main.tile([P, KT, NH * HD], F8)
        wk8_sb = main.tile([P, KT, NH * HD], F8)
        wv8_sb = main.tile([P, KT, NH * HD], F8)
        wp8_sb = main.tile([P, NH, D], F8)
        wp16_sb = main.tile([P, NH, D], BF)
        qT = main.tile([P, NH, S], BF)
        kT = main.tile([P, NH, S], BF)
        v8 = main.tile([P, KT, NH * HD], F8)
        v16 = main.tile([P, NH * HD], BF)
        a8 = main.tile([P, NH, S], F8)
        a16 = main.tile([P, NH, P], BF)

        # ---- input DMAs, ordered by first use across 4 queues ----
        # x (biggest, needed first): alternate chunks on sync/vector queues
        for kk in range(KT // 2):
            eng = nc.sync if kk % 2 == 0 else nc.gpsimd
            eng.dma_start(
                x8_sb[:, 2 * kk : 2 * kk + 2, :], x8[:, 2 * kk : 2 * kk + 2, :]
            )
        # scalar queue: first-use weights
        for h in range(2):
            cs = slice(h * HD, (h + 1) * HD)
            nc.scalar.dma_start(wk8_sb[:, :, cs], wk8[:, :, cs])
            nc.scalar.dma_start(wq8_sb[:, :, cs], wq8[:, :, cs])
        nc.scalar.dma_start(wv8_sb, wv8[:, :, :])
        for h in range(2, NH):
            cs = slice(h * HD, (h + 1) * HD)
            nc.scalar.dma_start(wk8_sb[:, :, cs], wk8[:, :, cs])
            nc.scalar.dma_start(wq8_sb[:, :, cs], wq8[:, :, cs])
        nc.scalar.dma_start(wp8_sb, wp8[:, :, :])
        nc.scalar.dma_start(wp16_sb, wp16[:, :, :])

        # ---- psum pools (16KB/partition budget: 8+4+2+2) ----
        pbig = top.enter_context(tc.tile_pool(name="pbig", bufs=2, space="PSUM"))
        psmall = top.enter_context(tc.tile_pool(name="psmall", bufs=2, space="PSUM"))
        upool = top.enter_context(tc.tile_pool(name="upool", bufs=1, space="PSUM"))
        dpool = top.enter_context(tc.tile_pool(name="dpool", bufs=1, space="PSUM"))

        def kq_unit(h, w_sb, out_sb, evac_engine):
            """Head h of the Q or K projection: out[hd, tok] in bf16 (scale 1/WS).

            W-stationary fp8 DR, 2 token-tiles per weight load so LDWEIGHTS
            stays off the critical path.
            """
            cs = slice(h * HD, (h + 1) * HD)
            for npass in range(2):
                ps = [psmall.tile([P, QW], F32, tag="ps", name=f"kqp{j}") for j in range(2)]
                for kk in range(KT // 2):
                    for j in range(2):
                        n = 2 * npass + j
                        nc.tensor.matmul(
                            ps[j],
                            lhsT=w_sb[:, 2 * kk : 2 * kk + 2, cs],
                            rhs=x8_sb[:, 2 * kk : 2 * kk + 2, n * QW : (n + 1) * QW],
                            start=(kk == 0),
                            stop=(kk == KT // 2 - 1),
                            perf_mode=DR,
                        )
                for j in range(2):
                    n = 2 * npass + j
                    evac_engine(out_sb[:, h, n * QW : (n + 1) * QW], ps[j])

        def act_evac_ws(out, ps):
            nc.scalar.activation(out, ps, COPY, scale=1.0 / WS)

        def dve_evac_ws(out, ps):
            nc.vector.tensor_scalar_mul(out, ps, 1.0 / WS)

        def v_unit(m):
            """Token-tile m of the V projection -> v8[:, m, :] fp8 (scale 1/WS)."""
            ps = psmall.tile([P, QW], F32, tag="ps")
            for kk in range(KT // 2):
                nc.tensor.matmul(
                    ps,
                    lhsT=x8_sb[:, 2 * kk : 2 * kk + 2, m * P : (m + 1) * P],
                    rhs=wv8_sb[:, 2 * kk : 2 * kk + 2, :],
                    start=(kk == 0),
                    stop=(kk == KT // 2 - 1),
                    perf_mode=DR,
                )
            nc.vector.tensor_scalar_mul(v8[:, m, :], ps, 1.0 / WS)

        epool = top.enter_context(tc.tile_pool(name="epool", bufs=3))
        e16pool = top.enter_context(tc.tile_pool(name="e16pool", bufs=2))
        ypool = top.enter_context(tc.tile_pool(name="ypool", bufs=2))
        rpool = top.enter_context(tc.tile_pool(name="rpool", bufs=2))
        rbpool = top.enter_context(tc.tile_pool(name="rbpool", bufs=2))

        def s_unit(h, qg):
            """Scores + exp for (head, q-group). Returns (e8 tile, e16 tile)."""
            e8t = epool.tile([P, KT, QW], F8, tag="e8", name=f"e8_{h}_{qg}")
            e16t = None
            npairs = 2 * qg + 2
            qs0 = qg * QW
            for t in range(npairs):
                diag1 = t == 2 * qg + 1
                c0 = 2 * P if diag1 else 0
                F = QW - c0
                sp = pbig.tile([P, 2, QW], F32, tag="sp", name=f"sp{h}{qg}{t}")
                for i in range(2):
                    kt = 2 * t + i
                    nc.tensor.matmul(
                        sp[:, i, 0:F],
                        lhsT=kT[:, h, kt * P : (kt + 1) * P],
                        rhs=qT[:, h, qs0 + c0 : qs0 + QW],
                        start=True,
                        stop=True,
                    )
                if t >= 2 * qg:
                    # diagonal pair: first kt gets a triangular mask block,
                    # second kt gets [full | triangular]
                    nc.vector.tensor_tensor(
                        sp[:, 0, 0:P], sp[:, 0, 0:P], mask_sb[:, 0:P], op=ADD
                    )
                    nc.vector.tensor_tensor(
                        sp[:, 1, 0 : 2 * P], sp[:, 1, 0 : 2 * P],
                        mask_sb[:, P : 3 * P], op=ADD,
                    )
                nc.scalar.activation(
                    e8t[:, 2 * t : 2 * t + 2, c0:QW], sp[:, :, 0:F], EXP, scale=SCALE
                )
                if qg == 0 and t == 0:
                    e16t = e16pool.tile([P, P], BF, tag="e16")
                    nc.scalar.activation(e16t, sp[:, 0, 0:P], EXP, scale=SCALE)
            return e8t, e16t

        def pv_unit(h, qg, e8t, e16t):
            """PV + denominators + normalize for (head, q-group) -> a8 / a16."""
            cs = slice(h * HD, (h + 1) * HD)
            npairs = 2 * qg + 2
            qs0 = qg * QW
            # denominators first: the rcp/broadcast chain runs on DVE/GpSimd
            # while the PE streams the PV matmuls, instead of serializing after
            dp = dpool.tile([1, QW], F32, tag="dp")
            for t in range(npairs):
                c0 = 2 * P if t == 2 * qg + 1 else 0
                nc.tensor.matmul(
                    dp[:, c0:],
                    lhsT=ones8,
                    rhs=e8t[:, 2 * t : 2 * t + 2, c0:],
                    start=(t == 0),
                    stop=(t == npairs - 1),
                    perf_mode=DR,
                )
            if qg == 0:
                # queries 0-127 attend only keys 0-127: bf16 denominator
                # (overwrite fp8 columns 0-127 of dp)
                nc.tensor.matmul(
                    dp[:, 0:P], lhsT=ones16, rhs=e16t, start=True, stop=True
                )
            rcp = rpool.tile([1, QW], F32, tag="rcp")
            nc.vector.reciprocal_approx_fast(rcp, dp)
            rb = rbpool.tile([P, QW], F32, tag="rb")
            nc.gpsimd.partition_broadcast(rb, rcp)
            up = upool.tile([P, QW], F32, tag="up")
            for t in range(npairs):
                c0 = 2 * P if t == 2 * qg + 1 else 0
                nc.tensor.matmul(
                    up[:, c0:],
                    lhsT=v8[:, 2 * t : 2 * t + 2, cs],
                    rhs=e8t[:, 2 * t : 2 * t + 2, c0:],
                    start=(t == 0),
                    stop=(t == npairs - 1),
                    perf_mode=DR,
                )
            upe = None
            if qg == 0:
                upe = psmall.tile([P, QW], F32, tag="ps", name=f"upe{h}")
                nc.tensor.matmul(
                    upe[:, 0:P], lhsT=v16[:, cs], rhs=e16t, start=True, stop=True
                )
            lo = P if qg == 0 else 0
            nc.vector.scalar_tensor_tensor(
                a8[:, h, qs0 + lo : qs0 + QW],
                up[:, lo:], AS, rb[:, lo:], op0=MULT, op1=MULT,
            )
            if qg == 0:
                nc.vector.tensor_tensor(
                    a16[:, h, :], upe[:, 0:P], rb[:, 0:P], op=MULT
                )

        def c_unit(qg):
            """Output projection + DMA for the 4 token-tiles of q-group qg."""
            for m in range(4 * qg, 4 * qg + 4):
                y_sb = ypool.tile([P, S], F32, tag="ysb", name=f"ysb{m}")
                if m == 0:
                    # bf16 path for tokens 0-127
                    for npb in range(2):
                        ps = pbig.tile([P, 2, QW], F32, tag="sp", name=f"ym0{npb}")
                        for h in range(NH):
                            for nl in range(2):
                                n = 2 * npb + nl
                                nc.tensor.matmul(
                                    ps[:, nl, :],
                                    lhsT=a16[:, h, :],
                                    rhs=wp16_sb[:, h, n * QW : (n + 1) * QW],
                                    start=(h == 0),
                                    stop=(h == NH - 1),
                                )
                        for nl in range(2):
                            n = 2 * npb + nl
                            nc.vector.tensor_copy(
                                y_sb[:, n * QW : (n + 1) * QW], ps[:, nl, :]
                            )
                else:
                    ms = slice(m * P, (m + 1) * P)
                    pss = [
                        pbig.tile([P, 2, QW], F32, tag="sp", name=f"yp{m}{j}")
                        for j in range(2)
                    ]
                    for hp in range(2):
                        for npb in range(2):
                            for nl in range(2):
                                n = 2 * npb + nl
                                nc.tensor.matmul(
                                    pss[npb][:, nl, :],
                                    lhsT=a8[:, 2 * hp : 2 * hp + 2, ms],
                                    rhs=wp8_sb[:, 2 * hp : 2 * hp + 2, n * QW : (n + 1) * QW],
                                    start=(hp == 0),
                                    stop=(hp == 1),
                                    perf_mode=DR,
                                )
                    for npb in range(2):
                        for nl in range(2):
                            n = 2 * npb + nl
                            # keep ACT free for the exp stream
                            nc.vector.tensor_scalar_mul(
                                y_sb[:, n * QW : (n + 1) * QW],
                                pss[npb][:, nl, :], 1.0 / (WS * AS),
                            )
                nc.sync.dma_start(y[m * P : (m + 1) * P, :], y_sb)

        # ---------------- main sequence ----------------
        with ExitStack() as pre:
            prepool = pre.enter_context(tc.tile_pool(name="prepool", bufs=1))
            wv16_sb = prepool.tile([P, KT, NH * HD], BF)
            xbT16_sb = prepool.tile([P, KT, P], BF)
            nc.gpsimd.dma_start(xbT16_sb, xbT16[:, :, :])
            nc.gpsimd.dma_start(wv16_sb, wv16[:, :, :])

            kq_unit(0, wk8_sb, kT, act_evac_ws)
            kq_unit(0, wq8_sb, qT, act_evac_ws)
            kq_unit(1, wk8_sb, kT, act_evac_ws)
            kq_unit(1, wq8_sb, qT, act_evac_ws)

            # bf16 V for keys 0-127
            psv = psmall.tile([P, QW], F32, tag="ps", name="psv16")
            for k in range(KT):
                nc.tensor.matmul(
                    psv,
                    lhsT=xbT16_sb[:, k, :],
                    rhs=wv16_sb[:, k, :],
                    start=(k == 0),
                    stop=(k == KT - 1),
                )
            nc.vector.tensor_copy(v16, psv)

        for qg in range(NQ):
            e_tiles = {}
            if qg == 0:
                e_tiles[0] = s_unit(0, 0)
                e_tiles[1] = s_unit(1, 0)
                kq_unit(2, wk8_sb, kT, dve_evac_ws)
                kq_unit(2, wq8_sb, qT, dve_evac_ws)
                for m in range(4):
                    v_unit(m)
                pv_unit(0, 0, *e_tiles[0])
                e_tiles[2] = s_unit(2, 0)
                kq_unit(3, wk8_sb, kT, dve_evac_ws)
                kq_unit(3, wq8_sb, qT, dve_evac_ws)
                pv_unit(1, 0, *e_tiles[1])
                e_tiles[3] = s_unit(3, 0)
                pv_unit(2, 0, *e_tiles[2])
                pv_unit(3, 0, *e_tiles[3])
            else:
                e_tiles[0] = s_unit(0, qg)
                e_tiles[1] = s_unit(1, qg)
                for m in range(4 * qg, 4 * qg + 4):
                    v_unit(m)
                pv_unit(0, qg, *e_tiles[0])
                e_tiles[2] = s_unit(2, qg)
                pv_unit(1, qg, *e_tiles[1])
                e_tiles[3] = s_unit(3, qg)
                pv_unit(2, qg, *e_tiles[2])
                pv_unit(3, qg, *e_tiles[3])
            c_unit(qg)

    nc.finalize()
    return nc


def _build_mask():
    # [tri | full | tri]: tri[p, c] = 0 where c >= p else MASK_NEG.
    # Applied pre-scale: exp(SCALE * (score + mask)).
    k = np.arange(P)[:, None]
    c = np.arange(P)[None, :]
    tri = np.where(c >= k, 0.0, MASK_NEG).astype(np.float32)
    full = np.full((P, P), MASK_NEG, dtype=np.float32)
    return np.concatenate([tri, full, tri], axis=1)


def _f8(a):
    return np.clip(a, -240.0, 240.0).astype(F8NP)


def _bf(a):
    return a.astype(BFNP)


def _pack_kps(mat_t, groups):
    """[rows=groups*128, cols] -> [128, groups, cols] with row = g*128+p."""
    r, c = mat_t.shape
    return np.ascontiguousarray(mat_t.reshape(groups, P, c).transpose(1, 0, 2))


_NC_CACHE = {}


def _get_nc():
    if "nc" not in _NC_CACHE:
        _NC_CACHE["nc"] = build_bass()
    return _NC_CACHE["nc"]


def make_in_maps(x, W_qkv, W_proj):
    x = np.asarray(x, dtype=np.float32)
    W_qkv = np.asarray(W_qkv, dtype=np.float32)
    W_proj = np.asarray(W_proj, dtype=np.float32)
    Wq, Wk, Wv = W_qkv[0:D], W_qkv[D : 2 * D], W_qkv[2 * D : 3 * D]
    mask = _build_mask()

    xb8 = []
    xbT = []
    for b in range(B):
        xT = x[b].T  # [D, S]
        xb8.append(_f8(_pack_kps(xT, KT)))
        xbT.append(_bf(_pack_kps(np.ascontiguousarray(xT[:, 0:P]), KT)))

    per_hg = []
    for hg in range(HG):
        rows = slice(hg * NH * HD, (hg + 1) * NH * HD)
        wq_t = Wq[rows].T  # [D, 512]
        wk_t = Wk[rows].T
        wv_t = Wv[rows].T
        wp_t = W_proj[:, rows].T  # [512, D]
        per_hg.append(
            {
                "wq8": _f8(_pack_kps(wq_t * WS, KT)),
                "wk8": _f8(_pack_kps(wk_t * WS, KT)),
                "wv8": _f8(_pack_kps(wv_t * WS, KT)),
                "wv16": _bf(_pack_kps(wv_t, KT)),
                "wp8": _f8(_pack_kps(wp_t * WS, NH)),
                "wp16": _bf(_pack_kps(wp_t, NH)),
            }
        )

    in_maps = []
    for c in range(8):
        b, hg = c // HG, c % HG
        m = {"x8": xb8[b], "xbT16": xbT[b], "mask": mask}
        m.update(per_hg[hg])
        in_maps.append(m)
    return in_maps


def run(x, W_qkv, W_proj, trace=False):
    nc = _get_nc()
    in_maps = make_in_maps(x, W_qkv, W_proj)
    res = run_bass_kernel_spmd(nc, in_maps, core_ids=list(range(8)), trace=trace)
    out = np.zeros((B, S, D), dtype=np.float32)
    for c in range(8):
        out[c // HG] += res.results[c]["y"]
    return out, res


def kernel(x, W_qkv, W_proj):
    out, _ = run(x, W_qkv, W_proj, trace=False)
    return out


# revision 16
# speedup vs baseline: 1.0289x; 1.0289x over previous
"""Causal self-attention (B=2, S=2048, D=2048, H=16) on 8 trn2 NeuronCores.

Sharding: core c -> batch b = c//4, head-group hg = c%4 (4 heads of 128 dims).
Each core computes its heads' attention plus the partial output projection
(row-parallel split of W_proj); the host sums the 4 partials per batch.

Mixed precision built around fp8e4 DoubleRow matmuls (2 fp8 weights per PE
cell -> 256-deep contraction at 0.5 cycles/row):
- Q/K/V projections: fp8 DR (weights prescaled x64, evac copy scales 1/64)
- scores: bf16 (same PE rate as fp32r, half the SBUF traffic)
- PV + softmax denominators: fp8 DR over e8 = exp tiles written as fp8
- output projection: fp8 DR (a prescaled x16, evac scales 1/1024)
Early rows are precision-critical (softmax concentrates on few keys), so
queries 0-127 (which only see keys 0-127) run a bf16 path end-to-end:
bf16 V for keys 0-127, bf16 e/ups/denominator, and a bf16 m-tile-0 output
projection. Everything stays in SBUF; no DRAM scratch round trips.
"""

import sys

sys.path.insert(0, "/opt/trn_rl_repo")

from contextlib import ExitStack

import numpy as np
import ml_dtypes

import concourse.bass as bass
import concourse.mybir as mybir
import concourse.tile as tile
from concourse import bacc
from concourse.bass_utils import run_bass_kernel_spmd

B, S, D, H = 2, 2048, 2048, 16
HD = D // H  # 128
NH = 4  # heads per core
HG = H // NH  # head groups = 4
P = 128
KT = D // P  # 16 k-tiles over model dim
NQ = 4  # q-groups of 512
QW = S // NQ  # 512
ST = S // P  # 16 token-tiles of 128
SCALE = float(1.0 / np.sqrt(D).astype(np.float32))
MASK_NEG = -30000.0  # exp(SCALE * -30000) == 0 in fp32
WS = 64.0  # weight prescale for fp8
AS = 16.0  # attention-out prescale for fp8

F32 = mybir.dt.float32
BF = mybir.dt.bfloat16
F8 = mybir.dt.float8e4
DR = mybir.MatmulPerfMode.DoubleRow
ADD = mybir.AluOpType.add
MULT = mybir.AluOpType.mult
EXP = mybir.ActivationFunctionType.Exp
COPY = mybir.ActivationFunctionType.Copy

F8NP = ml_dtypes.float8_e4m3
BFNP = ml_dtypes.bfloat16


def build_bass():
    nc = bacc.Bacc("TRN2")

    x8 = nc.declare_dram_parameter("x8", [P, KT, S], F8, isOutput=False)
    wq8 = nc.declare_dram_parameter("wq8", [P, KT, NH * HD], F8, isOutput=False)
    wk8 = nc.declare_dram_parameter("wk8", [P, KT, NH * HD], F8, isOutput=False)
    wv8 = nc.declare_dram_parameter("wv8", [P, KT, NH * HD], F8, isOutput=False)
    wv16 = nc.declare_dram_parameter("wv16", [P, KT, NH * HD], BF, isOutput=False)
    xbT16 = nc.declare_dram_parameter("xbT16", [P, KT, P], BF, isOutput=False)
    wp8 = nc.declare_dram_parameter("wp8", [P, NH, D], F8, isOutput=False)
    wp16 = nc.declare_dram_parameter("wp16", [P, NH, D], BF, isOutput=False)
    mask = nc.declare_dram_parameter("mask", [P, 3 * P], F32, isOutput=False)
    y = nc.declare_dram_parameter("y", [S, D], F32, isOutput=True)

    with tile.TileContext(nc) as tc, ExitStack() as top:
        const = top.enter_context(tc.tile_pool(name="const", bufs=1))
        main = top.enter_context(tc.tile_pool(name="main", bufs=1))

        # ---- constants ----
        mask_sb = const.tile([P, 3 * P], F32)
        nc.gpsimd.dma_start(mask_sb, mask[:, :])
        ones8_t = const.tile([P, 2, 16], F8)
        nc.vector.memset(ones8_t, 1.0)
        ones8 = ones8_t[:, :, 0:1]  # DR lhsT: pair step 16B (ISA: step%16==0)
        ones16_t = const.tile([P, 16], BF)
        nc.vector.memset(ones16_t, 1.0)
        ones16 = ones16_t[:, 0:1]

        # ---- persistent tensors ----
        x8_sb = main.tile([P, KT, S], F8)
        wq8_sb = main.tile([P, KT, NH * HD], F8)
        wk8_sb = main.tile([P, KT, NH * HD], F8)
        wv8_sb = main.tile([P, KT, NH * HD], F8)
        wp8_sb = main.tile([P, NH, D], F8)
        wp16_sb = main.tile([P, NH, D], BF)
        qT = main.tile([P, NH, S], BF)
        kT = main.tile([P, NH, S], BF)
        v8 = main.tile([P, KT, NH * HD], F8)
        v16 = main.tile([P, NH * HD], BF)
        a8 = main.tile([P, NH, S], F8)
        a16 = main.tile([P, NH, P], BF)

        # ---- input DMAs, ordered by first use across 4 queues ----
        # x (biggest, needed first): alternate chunks on sync/vector queues
        for kk in range(KT // 2):
            eng = nc.sync if kk % 2 == 0 else nc.gpsimd
            eng.dma_start(
                x8_sb[:, 2 * kk : 2 * kk + 2, :], x8[:, 2 * kk : 2 * kk + 2, :]
            )
        # scalar queue: first-use weights
        for h in range(2):
            cs = slice(h * HD, (h + 1) * HD)
            nc.scalar.dma_start(wk8_sb[:, :, cs], wk8[:, :, cs])
            nc.scalar.dma_start(wq8_sb[:, :, cs], wq8[:, :, cs])
        nc.scalar.dma_start(wv8_sb, wv8[:, :, :])
        for h in range(2, NH):
            cs = slice(h * HD, (h + 1) * HD)
            nc.scalar.dma_start(wk8_sb[:, :, cs], wk8[:, :, cs])
            nc.scalar.dma_start(wq8_sb[:, :, cs], wq8[:, :, cs])
        nc.scalar.dma_start(wp8_sb, wp8[:, :, :])
        nc.scalar.dma_start(wp16_sb, wp16[:, :, :])

        # ---- psum pools (16KB/partition budget: 8+4+4) ----
        pbig = top.enter_context(tc.tile_pool(name="pbig", bufs=2, space="PSUM"))
        psmall = top.enter_context(tc.tile_pool(name="psmall", bufs=2, space="PSUM"))
        # one [P, 2, QW] tile per pv_unit: slot 0 = ups, slot 1 row 0 = dps
        pvt = top.enter_context(tc.tile_pool(name="pvt", bufs=1, space="PSUM"))

        def kq_unit(h, w_sb, out_sb, evac_engine):
            """Head h of the Q or K projection: out[hd, tok] in bf16 (scale 1/WS).

            W-stationary fp8 DR, 4 token-tiles per weight load (2 pair-psums)
            so the 256-col DR LDWEIGHTS amortizes over 4 matmuls.
            """
            cs = slice(h * HD, (h + 1) * HD)
            pp = [pbig.tile([P, 2, QW], F32, tag="sp", name=f"kqp{j}") for j in range(2)]
            for kk in range(KT // 2):
                for n in range(4):
                    nc.tensor.matmul(
                        pp[n // 2][:, n % 2, :],
                        lhsT=w_sb[:, 2 * kk : 2 * kk + 2, cs],
                        rhs=x8_sb[:, 2 * kk : 2 * kk + 2, n * QW : (n + 1) * QW],
                        start=(kk == 0),
                        stop=(kk == KT // 2 - 1),
                        perf_mode=DR,
                    )
            for n in range(4):
                evac_engine(out_sb[:, h, n * QW : (n + 1) * QW], pp[n // 2][:, n % 2, :])

        def act_evac_ws(out, ps):
            nc.scalar.activation(out, ps, COPY, scale=1.0 / WS)

        def dve_evac_ws(out, ps):
            nc.vector.tensor_scalar_mul(out, ps, 1.0 / WS)

        def v_gen(m):
            """Token-tile m of the V projection -> v8[:, m, :] fp8 (scale 1/WS).

            Generator: yields est PE-ns after every 2 contraction pairs so the
            scheduler can weave it between exp-bound score pairs.
            """
            ps = psmall.tile([P, QW], F32, tag="ps", name=f"vps{m}")
            for kk in range(KT // 2):
                nc.tensor.matmul(
                    ps,
                    lhsT=x8_sb[:, 2 * kk : 2 * kk + 2, m * P : (m + 1) * P],
                    rhs=wv8_sb[:, 2 * kk : 2 * kk + 2, :],
                    start=(kk == 0),
                    stop=(kk == KT // 2 - 1),
                    perf_mode=DR,
                )
                if kk % 2 == 1:
                    yield 660
            nc.vector.tensor_scalar_mul(v8[:, m, :], ps, 1.0 / WS)
            yield 0

        epool = top.enter_context(tc.tile_pool(name="epool", bufs=3))
        e16pool = top.enter_context(tc.tile_pool(name="e16pool", bufs=2))
        ypool = top.enter_context(tc.tile_pool(name="ypool", bufs=2))
        rpool = top.enter_context(tc.tile_pool(name="rpool", bufs=2))
        rbpool = top.enter_context(tc.tile_pool(name="rbpool", bufs=2))

        def s_gen(h, qg, out):
            """Scores + exp for (head, q-group). Yields the exp-vs-PE deficit
            (ns of filler the scheduler should emit) after each pair.
            Appends (e8 tile, e16 tile) to `out`."""
            e8t = epool.tile([P, KT, QW], F8, tag="e8", name=f"e8_{h}_{qg}")
            e16t = None
            npairs = 2 * qg + 2
            qs0 = qg * QW
            for t in range(npairs):
                diag1 = t == 2 * qg + 1
                c0 = 2 * P if diag1 else 0
                F = QW - c0
                sp = pbig.tile([P, 2, QW], F32, tag="sp", name=f"sp{h}{qg}{t}")
                for i in range(2):
                    kt = 2 * t + i
                    nc.tensor.matmul(
                        sp[:, i, 0:F],
                        lhsT=kT[:, h, kt * P : (kt + 1) * P],
                        rhs=qT[:, h, qs0 + c0 : qs0 + QW],
                        start=True,
                        stop=True,
                    )
                if t >= 2 * qg:
                    # diagonal pair: first kt gets a triangular mask block,
                    # second kt gets [full | triangular]
                    nc.vector.tensor_tensor(
                        sp[:, 0, 0:P], sp[:, 0, 0:P], mask_sb[:, 0:P], op=ADD
                    )
                    nc.vector.tensor_tensor(
                        sp[:, 1, 0 : 2 * P], sp[:, 1, 0 : 2 * P],
                        mask_sb[:, P : 3 * P], op=ADD,
                    )
                nc.scalar.activation(
                    e8t[:, 2 * t : 2 * t + 2, c0:QW], sp[:, :, 0:F], EXP, scale=SCALE
                )
                deficit = 390 if diag1 else 600
                if qg == 0 and t == 0:
                    e16t = e16pool.tile([P, P], BF, tag="e16")
                    nc.scalar.activation(e16t, sp[:, 0, 0:P], EXP, scale=SCALE)
                    deficit += 280
                if t == npairs - 1:
                    out.append((e8t, e16t))
                yield deficit

        def pv_gen(h, qg, e8t, e16t):
            """PV + denominators + normalize for (head, q-group) -> a8 / a16.
            Generator chunks are ACT-lagged (their exps were issued one full
            S-unit earlier), so they serve as PE filler."""
            cs = slice(h * HD, (h + 1) * HD)
            npairs = 2 * qg + 2
            qs0 = qg * QW
            pt = pvt.tile([P, 2, QW], F32, tag="pv", name=f"pv{h}{qg}")
            up = pt[:, 0, :]
            dp = pt[0:1, 1, :]
            # denominators first so the rcp/broadcast chain overlaps the PV MMs
            for t in range(npairs):
                c0 = 2 * P if t == 2 * qg + 1 else 0
                nc.tensor.matmul(
                    dp[:, c0:],
                    lhsT=ones8,
                    rhs=e8t[:, 2 * t : 2 * t + 2, c0:],
                    start=(t == 0),
                    stop=(t == npairs - 1),
                    perf_mode=DR,
                )
                yield 130
            if qg == 0:
                # queries 0-127 attend only keys 0-127: bf16 denominator
                # (overwrite fp8 columns 0-127 of dp)
                nc.tensor.matmul(
                    dp[:, 0:P], lhsT=ones16, rhs=e16t, start=True, stop=True
                )
            rcp = rpool.tile([1, QW], F32, tag="rcp")
            nc.vector.reciprocal_approx_fast(rcp, dp)
            rb = rbpool.tile([P, QW], F32, tag="rb")
            nc.gpsimd.partition_broadcast(rb, rcp)
            yield 110
            for t in range(npairs):
                c0 = 2 * P if t == 2 * qg + 1 else 0
                nc.tensor.matmul(
                    up[:, c0:],
                    lhsT=v8[:, 2 * t : 2 * t + 2, cs],
                    rhs=e8t[:, 2 * t : 2 * t + 2, c0:],
                    start=(t == 0),
                    stop=(t == npairs - 1),
                    perf_mode=DR,
                )
                yield 330
            upe = None
            if qg == 0:
                upe = psmall.tile([P, QW], F32, tag="ps", name=f"upe{h}")
                nc.tensor.matmul(
                    upe[:, 0:P], lhsT=v16[:, cs], rhs=e16t, start=True, stop=True
                )
            lo = P if qg == 0 else 0
            nc.vector.scalar_tensor_tensor(
                a8[:, h, qs0 + lo : qs0 + QW],
                up[:, lo:], AS, rb[:, lo:], op0=MULT, op1=MULT,
            )
            if qg == 0:
                nc.vector.tensor_tensor(
                    a16[:, h, :], upe[:, 0:P], rb[:, 0:P], op=MULT
                )
            yield 130

        def c_unit(qg):
            """Output projection + DMA for the 4 token-tiles of q-group qg."""
            for m in range(4 * qg, 4 * qg + 4):
                y_sb = ypool.tile([P, S], F32, tag="ysb", name=f"ysb{m}")
                if m == 0:
                    # bf16 path for tokens 0-127
                    for npb in range(2):
                        ps = pbig.tile([P, 2, QW], F32, tag="sp", name=f"ym0{npb}")
                        for h in range(NH):
                            for nl in range(2):
                                n = 2 * npb + nl
                                nc.tensor.matmul(
                                    ps[:, nl, :],
                                    lhsT=a16[:, h, :],
                                    rhs=wp16_sb[:, h, n * QW : (n + 1) * QW],
                                    start=(h == 0),
                                    stop=(h == NH - 1),
                                )
                        for nl in range(2):
                            n = 2 * npb + nl
                            nc.vector.tensor_copy(
                                y_sb[:, n * QW : (n + 1) * QW], ps[:, nl, :]
                            )
                else:
                    ms = slice(m * P, (m + 1) * P)
                    pss = [
                        pbig.tile([P, 2, QW], F32, tag="sp", name=f"yp{m}{j}")
                        for j in range(2)
                    ]
                    for hp in range(2):
                        for npb in range(2):
                            for nl in range(2):
                                n = 2 * npb + nl
                                nc.tensor.matmul(
                                    pss[npb][:, nl, :],
                                    lhsT=a8[:, 2 * hp : 2 * hp + 2, ms],
                                    rhs=wp8_sb[:, 2 * hp : 2 * hp + 2, n * QW : (n + 1) * QW],
                                    start=(hp == 0),
                                    stop=(hp == 1),
                                    perf_mode=DR,
                                )
                    for npb in range(2):
                        for nl in range(2):
                            n = 2 * npb + nl
                            # keep ACT free for the exp stream
                            nc.vector.tensor_scalar_mul(
                                y_sb[:, n * QW : (n + 1) * QW],
                                pss[npb][:, nl, :], 1.0 / (WS * AS),
                            )
                nc.sync.dma_start(y[m * P : (m + 1) * P, :], y_sb)

        # ---------------- main sequence ----------------
        from collections import deque

        wv16_sb = main.tile([P, KT, NH * HD], BF)
        xbT16_sb = main.tile([P, KT, P], BF)
        nc.gpsimd.dma_start(xbT16_sb, xbT16[:, :, :])
        nc.gpsimd.dma_start(wv16_sb, wv16[:, :, :])

        # pre-phase: all 8 K/Q projection heads as one dense PE burst (warms
        # HAM; ACT only does the evac copies, the exp stream starts after)
        for h in range(NH):
            kq_unit(h, wk8_sb, kT, act_evac_ws if h < 2 else dve_evac_ws)
            kq_unit(h, wq8_sb, qT, act_evac_ws if h < 2 else dve_evac_ws)

        # bf16 V for keys 0-127
        psv = psmall.tile([P, QW], F32, tag="ps", name="psv16")
        for k in range(KT):
            nc.tensor.matmul(
                psv,
                lhsT=xbT16_sb[:, k, :],
                rhs=wv16_sb[:, k, :],
                start=(k == 0),
                stop=(k == KT - 1),
            )
        nc.vector.tensor_copy(v16, psv)

        def drain(gens, budget):
            """Run filler generators for ~budget ns of PE work."""
            while budget > 0 and gens:
                step = next(gens[0], None)
                if step is None:
                    gens.popleft()
                    continue
                budget -= max(step, 60)

        def drain_one(gen, budget):
            while budget > 0:
                step = next(gen, None)
                if step is None:
                    return
                budget -= max(step, 60)

        def run_all(gen):
            for _ in gen:
                pass

        # qg-cycles: scores feed ACT; V-projection tiles, the previous group's
        # output projection, and the previous head's PV fill the PE while the
        # exp stream catches up. Force-drain points keep the emission order
        # deadlock-free (a PE wait can only reference instructions already
        # emitted).
        for qg in range(NQ):
            vq = deque(v_gen(m) for m in range(4 * qg, 4 * qg + 4))
            e_tiles = []
            for deficit in s_gen(0, qg, e_tiles):
                drain(vq, deficit)
            for deficit in s_gen(1, qg, e_tiles):
                drain(vq, deficit)
            while vq:  # all of v8 must exist before any PV chunk runs
                run_all(vq.popleft())
            if qg > 0:
                c_unit(qg - 1)  # ACT-free block: exps of h0/h1 run ahead
            pv0 = pv_gen(0, qg, *e_tiles[0])
            for deficit in s_gen(2, qg, e_tiles):
                drain_one(pv0, deficit)
            run_all(pv0)  # e8 buf rotation: s_gen(3) reuses head-0's buffer
            pv1 = pv_gen(1, qg, *e_tiles[1])
            for deficit in s_gen(3, qg, e_tiles):
                drain_one(pv1, deficit)
            run_all(pv1)
            run_all(pv_gen(2, qg, *e_tiles[2]))
            run_all(pv_gen(3, qg, *e_tiles[3]))
        c_unit(NQ - 1)

    nc.finalize()
    return nc


def _build_mask():
    # [tri | full | tri]: tri[p, c] = 0 where c >= p else MASK_NEG.
    # Applied pre-scale: exp(SCALE * (score + mask)).
    k = np.arange(P)[:, None]
    c = np.arange(P)[None, :]
    tri = np.where(c >= k, 0.0, MASK_NEG).astype(np.float32)
    full = np.full((P, P), MASK_NEG, dtype=np.float32)
    return np.concatenate([tri, full, tri], axis=1)


def _f8(a):
    return np.clip(a, -240.0, 240.0).astype(F8NP)


def _bf(a):
    return a.astype(BFNP)


def _pack_kps(mat_t, groups):
    """[rows=groups*128, cols] -> [128, groups, cols] with row = g*128+p."""
    r, c = mat_t.shape
    return np.ascontiguousarray(mat_t.reshape(groups, P, c).transpose(1, 0, 2))


_NC_CACHE = {}


def _get_nc():
    if "nc" not in _NC_CACHE:
        _NC_CACHE["nc"] = build_bass()
    return _NC_CACHE["nc"]


def make_in_maps(x, W_qkv, W_proj):
    x = np.asarray(x, dtype=np.float32)
    W_qkv = np.asarray(W_qkv, dtype=np.float32)
    W_proj = np.asarray(W_proj, dtype=np.float32)
    Wq, Wk, Wv = W_qkv[0:D], W_qkv[D : 2 * D], W_qkv[2 * D : 3 * D]
    mask = _build_mask()

    xb8 = []
    xbT = []
    for b in range(B):
        xT = x[b].T  # [D, S]
        xb8.append(_f8(_pack_kps(xT, KT)))
        xbT.append(_bf(_pack_kps(np.ascontiguousarray(xT[:, 0:P]), KT)))

    per_hg = []
    for hg in range(HG):
        rows = slice(hg * NH * HD, (hg + 1) * NH * HD)
        wq_t = Wq[rows].T  # [D, 512]
        wk_t = Wk[rows].T
        wv_t = Wv[rows].T
        wp_t = W_proj[:, rows].T  # [512, D]
        per_hg.append(
            {
                "wq8": _f8(_pack_kps(wq_t * WS, KT)),
                "wk8": _f8(_pack_kps(wk_t * WS, KT)),
                "wv8": _f8(_pack_kps(wv_t * WS, KT)),
                "wv16": _bf(_pack_kps(wv_t, KT)),
                "wp8": _f8(_pack_kps(wp_t * WS, NH)),
                "wp16": _bf(_pack_kps(wp_t, NH)),
            }
        )

    in_maps = []
    for c in range(8):
        b, hg = c // HG, c % HG
        m = {"x8": xb8[b], "xbT16": xbT[b], "mask": mask}
        m.update(per_hg[hg])
        in_maps.append(m)
    return in_maps


def run(x, W_qkv, W_proj, trace=False):
    nc = _get_nc()
    in_maps = make_in_maps(x, W_qkv, W_proj)
    res = run_bass_kernel_spmd(nc, in_maps, core_ids=list(range(8)), trace=trace)
    out = np.zeros((B, S, D), dtype=np.float32)
    for c in range(8):
        out[c // HG] += res.results[c]["y"]
    return out, res


def kernel(x, W_qkv, W_proj):
    out, _ = run(x, W_qkv, W_proj, trace=False)
    return out
